# revision 1
# baseline (speedup 1.0000x reference)
"""DefocusBlur on 8 NeuronCores (Trainium2, Bass/Tile).

Depthwise 17x17 disk-blur of images [32,3,512,512] f32, reflect-101 pad.

Sharding: pure data parallel over batch — 4 images (12 planes) per core.

Per-core algorithm: the 2D conv is decomposed per kernel column j into a
1-D conv along H (as a PSUM-accumulated banded matmul, contraction over
128 padded input rows) with the W-shift j applied as a free-axis offset
into the W-padded input tile. The disk kernel is left-right symmetric
(kcol_j == kcol_{16-j}), so mirror pairs are pre-summed on the vector
engine and share one matmul each: all 4 (or 5, on 3 of 4 blocks) pair
sums are computed by ONE wide DVE op using overlapping-window 3D APs
(in0 stride +1 from col 0, in1 stride -1 from col 16), amortizing the
per-op overhead; the promote fraction 42/57 is the PE/DVE balance
point for the fused-add cost. Inputs are
reflect-padded by 8 on the host so no edge logic runs on device.
Matmuls run as float32r (full PE rate at N=512, ~1e-4 rel err).

Schedule details (from cost-model trace analysis): all single-column
matmuls issue first, pair-matmuls last (with the alternating 4/5 pairing
and triple-buffered s-tiles, DVE runs ahead so PE never waits); weights
load as per-group chunks on the scalar-engine HWDGE ring (parallel to
input loads on the sync ring) ordered by first use; output DMAs also
ride the scalar ring; a short dummy-matmul stream warms the PE clock
gate (HAM) during the initial DMA wait. All 12 padded planes are
processed as one flat 6336-row space with M=112 blocks tiled across
plane boundaries (banded weights are translation-invariant); the
16-row pad-seam outputs are computed but never stored.
"""
import dataclasses

import numpy as np

_RADIUS = 8
_B, _C, _H, _W = 32, 3, 512, 512
_NCORES = 8
_PLANES = (_B // _NCORES) * _C
_M = 112
_KIN = _M + 2 * _RADIUS
_NBLK = 5
_HP = _H + 2 * _RADIUS
_WP = _W + 2 * _RADIUS

NPAIR = 4  # pairs pre-summed on DVE; groups = 17 - NPAIR


def _disk_kernel():
    L = np.arange(-8, 9)
    X, Y = np.meshgrid(L, L)
    disk = ((X ** 2 + Y ** 2) <= _RADIUS ** 2).astype(np.float32)
    disk /= disk.sum()
    x = np.arange(3, dtype=np.float32) - 1
    g = np.exp(-(x ** 2) / (2.0 * 0.5 ** 2))
    g /= g.sum()
    k2 = np.outer(g, g).astype(np.float32)
    p = np.pad(disk, 1, mode="reflect")
    out = np.zeros_like(disk)
    for i in range(3):
        for j in range(3):
            out += k2[i, j] * p[i : i + 17, j : j + 17]
    return out


def _groups():
    """Returns list of (cols, kcol_index): cols = list of W-shifts sharing
    banded weight kcol_index."""
    gs = []
    for j in range(NPAIR):
        gs.append(([j, 16 - j], j))
    for j in range(NPAIR, 17 - NPAIR):
        gs.append(([j], j))
    return gs


def _banded_weights():
    k2d = _disk_kernel()
    ws = []
    for _, j in _groups():
        B = np.zeros((_KIN, _M), np.float32)
        for c in range(_M):
            B[c : c + 17, c] = k2d[:, j]
        ws.append(B)
    return np.ascontiguousarray(np.concatenate(ws, axis=1))


_NC_CACHE = []


def _build_program():
    import concourse.bacc as bacc
    import concourse.mybir as mybir
    import concourse.tile as tile

    f32 = mybir.dt.float32
    f32r = mybir.dt.float32r
    gs = _groups()
    ng = len(gs)

    nc = bacc.Bacc("TRN2", target_bir_lowering=False, debug=False)
    x_d = nc.dram_tensor("x", [_PLANES, _HP, _WP], f32r, kind="ExternalInput")
    w_d = nc.dram_tensor("w", [_KIN, ng * _M], f32r, kind="ExternalInput")
    o_d = nc.dram_tensor("o", [_PLANES, _H, _W], f32, kind="ExternalOutput")

    with tile.TileContext(nc) as tc:
        with (
            tc.tile_pool(name="wpool", bufs=1) as wpool,
            tc.tile_pool(name="inp", bufs=3) as inp,
            tc.tile_pool(name="spool", bufs=4) as spool,
            tc.tile_pool(name="outp", bufs=3) as outp,
            tc.tile_pool(name="ps", bufs=3, space="PSUM") as psp,
        ):
            wt = wpool.tile([_KIN, ng * _M], f32r)
            # HAM warm-up: keep PE busy during the initial DMA wait so the
            # first real matmuls run at full clock.
            warm = wpool.tile([128, 64], f32)
            nc.gpsimd.memset(warm[:], 0.0)
            wps = psp.tile([64, 64], f32, tag="warm")
            for wi in range(12):
                nc.tensor.matmul(
                    wps[:], warm[:, :64], warm[:, :64],
                    start=(wi == 0), stop=(wi == 11),
                )
            w_loaded = [False]
            GH = _PLANES * _HP              # 6336 global padded rows
            NSTART = GH - 2 * _RADIUS       # 6320 window starts
            xf = x_d.rearrange("p h w -> (p h) w")
            nblocks = (NSTART + _M - 1) // _M
            for b in range(nblocks):
                    g0 = b * _M
                    mb = min(_M, NSTART - g0)
                    kb = mb + 2 * _RADIUS
                    xt = inp.tile([_KIN, _WP], f32r, tag="xt")
                    nc.sync.dma_start(xt[:kb, :], xf[g0 : g0 + kb, :])
                    if not w_loaded[0]:
                        w_loaded[0] = True
                        worder = (
                            [4] + [0] + [5, 6] + [1] + [7, 8] + [2]
                            + [9, 10, 11, 12] + [3]
                        )
                        for wg in worder:
                            nc.scalar.dma_start(
                                wt[:, wg * _M : (wg + 1) * _M],
                                w_d[:, wg * _M : (wg + 1) * _M],
                            )
                    promote = (b % 4 != 0)
                    npr = 5 if promote else 4
                    st4 = spool.tile([_KIN, 5, _W], f32r, tag="st4")
                    full = xt[:kb, :]
                    pdim = list(full.ap[0])
                    in0 = dataclasses.replace(
                        full, ap=[pdim, [1, npr], [1, _W]]
                    )
                    in1 = dataclasses.replace(
                        full, offset=full.offset + 16,
                        ap=[pdim, [-1, npr], [1, _W]],
                    )
                    nc.vector.tensor_add(st4[:kb, :npr], in0, in1)
                    ps = psp.tile([_M, _W], f32, tag="ps")
                    singles = [gi for gi, (c, _) in enumerate(gs) if len(c) == 1]
                    pairs = [gi for gi, (c, _) in enumerate(gs) if len(c) == 2]
                    order = singles + pairs
                    mms = []
                    for gi in order:
                        cols, _ = gs[gi]
                        if promote and gi == 4:
                            continue
                        elif promote and gi == 12:
                            continue
                        elif len(cols) == 1:
                            mms.append((gi, xt[:kb, cols[0] : cols[0] + _W]))
                        else:
                            mms.append((gi, st4[:kb, gi, :]))
                    if promote:
                        mms.append((4, st4[:kb, 4, :]))
                    for mi, (gi, rhs) in enumerate(mms):
                        nc.tensor.matmul(
                            ps[:mb, :],
                            wt[:kb, gi * _M : gi * _M + mb],
                            rhs,
                            start=(mi == 0),
                            stop=(mi == len(mms) - 1),
                        )
                    ot = outp.tile([_M, _W], f32, tag="ot")
                    nc.scalar.copy(ot[:mb, :], ps[:mb, :])
                    # store only valid output runs (skip pad-seam rows)
                    for p in range(_PLANES):
                        lo = max(g0, p * _HP)
                        hi = min(g0 + mb, p * _HP + _H)
                        if lo < hi:
                            nc.scalar.dma_start(
                                o_d[p, lo - p * _HP : hi - p * _HP, :],
                                ot[lo - g0 : hi - g0, :],
                            )
    nc.compile()
    return nc


def _get_program():
    if not _NC_CACHE:
        _NC_CACHE.append(_build_program())
    return _NC_CACHE[0]


def kernel(images: np.ndarray) -> np.ndarray:
    from concourse.bass_utils import run_bass_kernel_spmd

    images = np.asarray(images, dtype=np.float32)
    padded = np.pad(
        images, ((0, 0), (0, 0), (_RADIUS, _RADIUS), (_RADIUS, _RADIUS)),
        mode="reflect",
    )
    shards = np.ascontiguousarray(padded.reshape(_NCORES, _PLANES, _HP, _WP))
    w = _banded_weights()
    nc = _get_program()
    in_maps = [{"x": shards[c], "w": w} for c in range(_NCORES)]
    res = run_bass_kernel_spmd(nc, in_maps, list(range(_NCORES)))
    out = np.stack([res.results[c]["o"] for c in range(_NCORES)], axis=0)
    return np.ascontiguousarray(out.reshape(_B, _C, _H, _W).astype(np.float32))



# revision 2
# speedup vs baseline: 2.6881x; 2.6881x over previous
"""DefocusBlur on 8 NeuronCores (Trainium2, Bass/Tile).

Depthwise 17x17 disk-blur of images [32,3,512,512] f32, reflect-101 pad.

Sharding: pure data parallel over batch — 4 images (12 planes) per core.

Per-core algorithm (fp8 DoubleRow rewrite of the banded-matmul scheme):
the 2D conv is decomposed per kernel column j into a 1-D conv along H
(a PSUM-accumulated banded matmul, contraction over 128 padded input
rows) with the W-shift j applied as a free-axis offset into the
W-padded input tile. Inputs are quantized to fp8 e4m3 on the host
(output error ~5e-3 « the 2e-2 gate); weights are quantized to fp8
scaled by 128 (to clear the e4m3 denormal floor) with error-feedback
rounding per band column, and the 1/128 descale rides the PSUM->SBUF
copy on the scalar engine (Activation Copy with scale), which also
narrows to bf16 to halve the store traffic.

fp8 matmuls run in MatmulPerfMode.DoubleRow: each instruction carries
TWO [128]-contraction k-tiles (0.5 cycles/row — 4x the fp32r MAC
rate), so two kernel columns (two W-shifts of the same tile, one
strided 3-D AP) share one matmul. The mirror pair col0+col16 is
pre-summed on the (otherwise idle) vector engine into a tail region of
the input tile — same tile, so it pairs with col8 in one DoubleRow AP
— leaving 16 operands = 8 matmuls per 112-row block.

DMA instruction count is minimized (the DGE is a serial resource):
input loads and output stores are batched 8 blocks per DMA (3-D APs
over a [128, 8, 1040] input / [112, 8, 512] output supertile); weights
load in one DMA. All 12 padded planes are processed as one flat
6336-row space (banded weights are translation-invariant); block rows
that fall in the 16-row pad seams are computed but sliced off on the
host. A dummy-matmul stream warms the PE clock (p-state ramp) during
the initial DMA wait.
"""
import dataclasses

import numpy as np

_RADIUS = 8
_B, _C, _H, _W = 32, 3, 512, 512
_NCORES = 8
_PLANES = (_B // _NCORES) * _C
_M = 112
_KIN = _M + 2 * _RADIUS
_HP = _H + 2 * _RADIUS
_WP = _W + 2 * _RADIUS

_GH = _PLANES * _HP            # 6336 flat padded rows per core
_NSTART = _GH - 2 * _RADIUS    # 6320 valid window starts
_NBLOCKS = (_NSTART + _M - 1) // _M   # 57
_SB = 8                        # blocks per DMA supertile
_NSUPER = (_NBLOCKS + _SB - 1) // _SB  # 8 (7 full + 1x1)
_XROWS = (_NBLOCKS - 1) * _M + _KIN    # 6400 padded input rows
_OROWS = _NBLOCKS * _M                 # 6384 output rows (tail garbage)

_SLOT = 1040                   # 528 input cols + 512 pair-sum cols
_WSCALE = 128.0

# operand pairs per DoubleRow matmul: offsets into a slot's 1040 cols.
# 528 = the DVE pair-sum s0 = x[:, 0:512] + x[:, 16:528] (band k0).
# band index for offset o: o == 528 -> 0 else min(o, 16 - o).
_PAIRS = [(1, 15), (2, 14), (3, 13), (4, 12), (5, 11), (6, 10), (7, 9),
          (528, 8)]


def _disk_kernel():
    L = np.arange(-8, 9)
    X, Y = np.meshgrid(L, L)
    disk = ((X ** 2 + Y ** 2) <= _RADIUS ** 2).astype(np.float32)
    disk /= disk.sum()
    x = np.arange(3, dtype=np.float32) - 1
    g = np.exp(-(x ** 2) / (2.0 * 0.5 ** 2))
    g /= g.sum()
    k2 = np.outer(g, g).astype(np.float32)
    p = np.pad(disk, 1, mode="reflect")
    out = np.zeros_like(disk)
    for i in range(3):
        for j in range(3):
            out += k2[i, j] * p[i : i + 17, j : j + 17]
    return out


def _fp8_quantize_column(col, f8dt):
    """Round a 17-tap column to fp8 with error-feedback so the column sum
    stays tight (keeps the DC gain of the blur accurate)."""
    out = np.zeros_like(col)
    carry = 0.0
    for i in range(col.shape[0]):
        want = col[i] + carry
        q = float(np.asarray(want, dtype=np.float32).astype(f8dt))
        out[i] = q
        carry = want - q
    return out


def _banded_weights():
    """[KIN, 8, 2, M] fp8: band for _PAIRS[g][i] at [:, g, i, :]."""
    import ml_dtypes

    f8 = ml_dtypes.float8_e4m3
    k2d = _disk_kernel() * _WSCALE
    cols = {}
    for j in range(9):
        cols[j] = _fp8_quantize_column(k2d[:, j].astype(np.float64), f8)
    w = np.zeros((_KIN, len(_PAIRS), 2, _M), np.float32)
    for g, (a, b) in enumerate(_PAIRS):
        for i, o in enumerate((a, b)):
            band = cols[0 if o == 528 else min(o, 16 - o)]
            for m in range(_M):
                w[m : m + 17, g, i, m] = band
    return np.ascontiguousarray(w.astype(f8))


_NC_CACHE = []


def _build_program():
    import concourse.bacc as bacc
    import concourse.mybir as mybir
    import concourse.tile as tile

    f32 = mybir.dt.float32
    f8 = mybir.dt.float8e4
    bf16 = mybir.dt.bfloat16
    DR = mybir.MatmulPerfMode.DoubleRow
    ng = len(_PAIRS)

    nc = bacc.Bacc("TRN2", target_bir_lowering=False, debug=False)
    x_d = nc.dram_tensor("x", [_XROWS, _WP], f8, kind="ExternalInput")
    w_d = nc.dram_tensor("w", [_KIN, ng, 2, _M], f8, kind="ExternalInput")
    o_d = nc.dram_tensor("o", [_OROWS, _W], bf16, kind="ExternalOutput")

    with tile.TileContext(nc) as tc:
        with (
            tc.tile_pool(name="wpool", bufs=1) as wpool,
            tc.tile_pool(name="inp", bufs=2) as inp,
            tc.tile_pool(name="outp", bufs=2) as outp,
            tc.tile_pool(name="ps", bufs=4, space="PSUM") as psp,
        ):
            wt = wpool.tile([_KIN, ng, 2, _M], f8)
            # HAM warm-up: keep PE busy during the initial DMA wait so the
            # first real matmuls run at full clock.
            warm = wpool.tile([128, 64], f32)
            nc.gpsimd.memset(warm[:], 0.0)
            wps = psp.tile([64, 64], f32, tag="warm")
            for wi in range(12):
                nc.tensor.matmul(
                    wps[:], warm[:, :64], warm[:, :64],
                    start=(wi == 0), stop=(wi == 11),
                )
            nc.scalar.dma_start(wt[:], w_d[:])
            for sb in range(_NSUPER):
                nblk = min(_SB, _NBLOCKS - sb * _SB)
                g0 = sb * _SB * _M
                xt = inp.tile([_KIN, _SB, _SLOT], f8, tag="xt")
                # one DMA: nblk blocks of 128 rows (stride 112 rows apart)
                dst = xt[:, :nblk, : _WP]
                src1 = x_d[g0 : g0 + _KIN, :]
                src = dataclasses.replace(
                    src1,
                    ap=[list(src1.ap[0]), [_M * _WP, nblk], [1, _WP]],
                )
                nc.sync.dma_start(dst, src)
                ot = outp.tile([_M, _SB, _W], bf16, tag="ot")
                for i in range(nblk):
                    slot = xt[:, i, :]
                    # DVE pair-sum s0 = col0 + col16 into the slot tail
                    nc.vector.tensor_add(
                        xt[:, i, _WP : _WP + _W],
                        xt[:, i, 0:_W],
                        xt[:, i, 2 * _RADIUS : 2 * _RADIUS + _W],
                    )
                    ps = psp.tile([_M, _W], f32, tag="ps")
                    pdim = list(slot.ap[0])
                    for g, (a, b) in enumerate(_PAIRS):
                        rhs = dataclasses.replace(
                            slot,
                            offset=slot.offset + a,
                            ap=[pdim, [b - a, 2], [1, _W]],
                        )
                        nc.tensor.matmul(
                            ps[:], wt[:, g, :, :], rhs,
                            start=(g == 0), stop=(g == ng - 1),
                            perf_mode=DR,
                        )
                    nc.scalar.mul(ot[:, i, :], ps[:], 1.0 / _WSCALE)
                # one DMA: store nblk blocks (output rows are block-major)
                osrc = ot[:, :nblk, :]
                od1 = o_d[g0 : g0 + _M, :]
                odst = dataclasses.replace(
                    od1,
                    ap=[list(od1.ap[0]), [_M * _W, nblk], [1, _W]],
                )
                nc.scalar.dma_start(odst, osrc)
    nc.compile()
    return nc


def _get_program():
    if not _NC_CACHE:
        _NC_CACHE.append(_build_program())
    return _NC_CACHE[0]


def kernel(images: np.ndarray) -> np.ndarray:
    import ml_dtypes
    from concourse.bass_utils import run_bass_kernel_spmd

    f8 = ml_dtypes.float8_e4m3
    images = np.asarray(images, dtype=np.float32)
    padded = np.pad(
        images, ((0, 0), (0, 0), (_RADIUS, _RADIUS), (_RADIUS, _RADIUS)),
        mode="reflect",
    )
    shards = padded.reshape(_NCORES, _PLANES * _HP, _WP)
    xs = np.zeros((_NCORES, _XROWS, _WP), dtype=f8)
    xs[:, : _PLANES * _HP, :] = shards.astype(f8)
    w = _banded_weights()
    nc = _get_program()
    in_maps = [{"x": xs[c], "w": w} for c in range(_NCORES)]
    res = run_bass_kernel_spmd(nc, in_maps, list(range(_NCORES)))
    out = np.stack(
        [np.asarray(res.results[c]["o"]) for c in range(_NCORES)], axis=0
    )
    out = out[:, : _PLANES * _HP, :].astype(np.float32)
    out = out.reshape(_NCORES, _PLANES, _HP, _W)[:, :, : _H, :]
    return np.ascontiguousarray(
        out.reshape(_B, _C, _H, _W).astype(np.float32)
    )


# revision 20
# speedup vs baseline: 2.8211x; 1.0495x over previous
"""DefocusBlur on 8 NeuronCores (Trainium2, Bass/Tile).

Depthwise 17x17 disk-blur of images [32,3,512,512] f32, reflect-101 pad.

Sharding: pure data parallel over batch — 4 images (12 planes) per core.

Per-core algorithm (fp8 DoubleRow rewrite of the banded-matmul scheme):
the 2D conv is decomposed per kernel column j into a 1-D conv along H
(a PSUM-accumulated banded matmul, contraction over 128 padded input
rows) with the W-shift j applied as a free-axis offset into the
W-padded input tile. fp8 matmuls run in MatmulPerfMode.DoubleRow: each
instruction carries TWO [128]-contraction k-tiles (0.5 cycles/row — 4x
the fp32r MAC rate), so two kernel columns (two W-shifts of the same
tile, one strided 3-D AP) share one matmul. The mirror pair
col0+col16 is pre-summed on the (otherwise idle) vector engine into a
tail region of the input tile — same tile, so it pairs with col8 in
one DoubleRow AP — leaving 16 operands = 8 matmuls per 112-row block.

Numerics (rel-err budget ~5e-3 vs the 2e-2 gate): inputs are
quantized to fp8 e4m3 on the host with error-diffusion rounding along
W (the conv's windowed sums see ~3x less quantization error than
round-to-nearest); weights are scaled by 246 — placing the dominant
disk weight near the top of a binade, found by sweep — and quantized
with error feedback per band column; the 1/246 descale rides the
PSUM->SBUF copy on the scalar engine (Activation Copy with scale),
which narrows to fp16 (not bf16: 3 extra mantissa bits, same DMA
cost).

DMA instruction count is minimized (HWDGE is a serial ~640ns/DMA
resource): input loads and output stores are batched per supertile
(3-D APs; supertile sizes [2,8,...,8,6,1] — small first for a fast
pipeline fill, tiny last for a short drain tail); weights load in one
DMA. All 12 padded planes are processed as one flat 6336-row space
(banded weights are translation-invariant); block rows that fall in
the 16-row pad seams are computed but sliced off on the host. A
dummy-matmul stream warms the PE clock (p-state ramp) during the
initial DMA wait and chains gaplessly into the real matmuls.
"""
import dataclasses

import numpy as np

_RADIUS = 8
_B, _C, _H, _W = 32, 3, 512, 512
_NCORES = 8
_PLANES = (_B // _NCORES) * _C
_M = 112
_KIN = _M + 2 * _RADIUS
_HP = _H + 2 * _RADIUS
_WP = _W + 2 * _RADIUS

_GH = _PLANES * _HP            # 6336 flat padded rows per core
_NSTART = _GH - 2 * _RADIUS    # 6320 valid window starts
_NBLOCKS = (_NSTART + _M - 1) // _M   # 57
_SB = 8                        # max blocks per DMA supertile
_SUPERS = [2, 4, 6, 8, 8, 8, 8, 8, 4, 1]       # sums to 57; ramped start
_XROWS = (_NBLOCKS - 1) * _M + _KIN    # 6400 padded input rows
_OROWS = _NBLOCKS * _M                 # 6384 output rows (tail garbage)

_SLOT = 1040                   # 528 input cols + 512 pair-sum cols
_WSCALE = 246.0
_NWARM = 12

# operand pairs per DoubleRow matmul: offsets into a slot's 1040 cols.
# 528 = the DVE pair-sum s0 = x[:, 0:512] + x[:, 16:528] (band k0).
# band index for offset o: o == 528 -> 0 else min(o, 16 - o).
_PAIRS = [(1, 15), (2, 14), (3, 13), (4, 12), (5, 11), (6, 10), (7, 9),
          (528, 8)]


def _disk_kernel():
    L = np.arange(-8, 9)
    X, Y = np.meshgrid(L, L)
    disk = ((X ** 2 + Y ** 2) <= _RADIUS ** 2).astype(np.float32)
    disk /= disk.sum()
    x = np.arange(3, dtype=np.float32) - 1
    g = np.exp(-(x ** 2) / (2.0 * 0.5 ** 2))
    g /= g.sum()
    k2 = np.outer(g, g).astype(np.float32)
    p = np.pad(disk, 1, mode="reflect")
    out = np.zeros_like(disk)
    for i in range(3):
        for j in range(3):
            out += k2[i, j] * p[i : i + 17, j : j + 17]
    return out


def _fp8_quantize_column(col, f8dt):
    """Round a 17-tap column to fp8 with error-feedback so the column sum
    stays tight (keeps the DC gain of the blur accurate)."""
    out = np.zeros_like(col)
    carry = 0.0
    for i in range(col.shape[0]):
        want = col[i] + carry
        q = float(np.asarray(want, dtype=np.float32).astype(f8dt))
        out[i] = q
        carry = want - q
    return out


def _banded_weights():
    """[KIN, 8, 2, M] fp8: band for _PAIRS[g][i] at [:, g, i, :]."""
    import ml_dtypes

    f8 = ml_dtypes.float8_e4m3
    k2d = _disk_kernel().astype(np.float64) * _WSCALE
    cols = {}
    for j in range(9):
        cols[j] = _fp8_quantize_column(k2d[:, j], f8)
    w = np.zeros((_KIN, len(_PAIRS), 2, _M), np.float32)
    for g, (a, b) in enumerate(_PAIRS):
        for i, o in enumerate((a, b)):
            band = cols[0 if o == 528 else min(o, 16 - o)]
            for m in range(_M):
                w[m : m + 17, g, i, m] = band
    return np.ascontiguousarray(w.astype(f8))


def _quantize_diffuse(a, f8dt):
    """fp8-quantize along the last axis with 1-D error diffusion: windowed
    sums (what the conv computes) see only the boundary carries instead of
    289 independent rounding errors."""
    a = a.astype(np.float32)
    q = np.empty(a.shape, dtype=f8dt)
    carry = np.zeros(a.shape[:-1], np.float32)
    for w in range(a.shape[-1]):
        want = a[..., w] + carry
        qq = want.astype(f8dt)
        q[..., w] = qq
        carry = want - qq.astype(np.float32)
    return q


_NC_CACHE = []


def _build_program(supers=None, nwarm=None, w_pool_ring=False, store_per=2,
                   outp_bufs=4, final_psum_store=False):
    import concourse.bacc as bacc
    import concourse.mybir as mybir
    import concourse.tile as tile

    supers = supers or _SUPERS
    nwarm = nwarm or _NWARM
    f32 = mybir.dt.float32
    f8 = mybir.dt.float8e4
    f16 = mybir.dt.float16
    DR = mybir.MatmulPerfMode.DoubleRow
    ng = len(_PAIRS)
    assert sum(supers) == _NBLOCKS

    nc = bacc.Bacc("TRN2", target_bir_lowering=False, debug=False)
    x_d = nc.dram_tensor("x", [_XROWS, _WP], f8, kind="ExternalInput")
    w_d = nc.dram_tensor("w", [_KIN, ng, 2, _M], f8, kind="ExternalInput")
    o_d = nc.dram_tensor("o", [_OROWS, _W], f16, kind="ExternalOutput")
    o2_d = None
    if final_psum_store:
        # final 48 valid rows ride straight from PSUM (f32, still x_WSCALE):
        # skips the scalar-engine copy on the drain tail; host descales.
        o2_d = nc.dram_tensor("o2", [48, _W], f32, kind="ExternalOutput")

    with tile.TileContext(nc) as tc:
        with (
            tc.tile_pool(name="wpool", bufs=1) as wpool,
            tc.tile_pool(name="inp", bufs=3) as inp,
            tc.tile_pool(name="outp", bufs=outp_bufs) as outp,
            tc.tile_pool(name="ps", bufs=6, space="PSUM") as psp,
            tc.tile_pool(name="psw", bufs=1, space="PSUM") as psw,
        ):
            wt = wpool.tile([_KIN, ng, 2, _M], f8)
            # HAM warm-up: keep PE busy during the initial DMA wait so the
            # first real matmuls run at full clock.
            warm = wpool.tile([128, 64], f32)
            nc.gpsimd.memset(warm[:], 0.0)
            wps = psw.tile([64, 64], f32, tag="warm")
            for wi in range(nwarm):
                nc.tensor.matmul(
                    wps[:], warm[:, :64], warm[:, :64],
                    start=(wi == 0), stop=(wi == nwarm - 1),
                )
            # weights ride the Pool (SWDGE) ring: skips the serial HWDGE
            # slot so the startup input DMAs aren't queued behind it.
            if w_pool_ring:
                nc.gpsimd.dma_start(wt[:], w_d[:])
            else:
                nc.scalar.dma_start(wt[:], w_d[:])
            sup_of = {}
            s0 = 0
            for n in supers:
                sup_of[s0] = n
                s0 += n
            sup_base = 0
            xt = None
            ot = None
            for b in range(_NBLOCKS):
                if b in sup_of:
                    nsup = sup_of[b]
                    sup_base = b
                    g0 = b * _M
                    xt = inp.tile([_KIN, _SB, _SLOT], f8, tag="xt")
                    # one DMA: nsup blocks of 128 rows, 112 rows apart
                    dst = xt[:, :nsup, : _WP]
                    src1 = x_d[g0 : g0 + _KIN, :]
                    src = dataclasses.replace(
                        src1,
                        ap=[list(src1.ap[0]), [_M * _WP, nsup], [1, _WP]],
                    )
                    nc.sync.dma_start(dst, src)
                i = b - sup_base
                slot = xt[:, i, :]
                # DVE pair-sum s0 = col0 + col16 into the slot tail
                nc.vector.tensor_add(
                    xt[:, i, _WP : _WP + _W],
                    xt[:, i, 0:_W],
                    xt[:, i, 2 * _RADIUS : 2 * _RADIUS + _W],
                )
                ps = psp.tile([_M, _W], f32, tag="ps")
                pdim = list(slot.ap[0])
                for g, (a, bb) in enumerate(_PAIRS):
                    rhs = dataclasses.replace(
                        slot,
                        offset=slot.offset + a,
                        ap=[pdim, [bb - a, 2], [1, _W]],
                    )
                    nc.tensor.matmul(
                        ps[:], wt[:, g, :, :], rhs,
                        start=(g == 0), stop=(g == ng - 1),
                        perf_mode=DR,
                    )
                sp = store_per
                if final_psum_store and b == _NBLOCKS - 1:
                    nv = _NSTART - b * _M
                    nc.sync.dma_start(o2_d[:, :], ps[:nv, :])
                    continue
                if b % sp == 0:
                    ot = outp.tile([_M, sp, _W], f16, tag="ot")
                nc.scalar.mul(ot[:, b % sp, :], ps[:], 1.0 / _WSCALE)
                # store every `sp` blocks (output rows are block-major), so
                # transfers pipeline on the serial DMA-engines slot instead
                # of bunching behind a whole supertile's copies.
                ns = b % sp + 1
                if ns == sp or b == _NBLOCKS - 1:
                    bb0 = b - ns + 1
                    if b == _NBLOCKS - 1:
                        # final block: only 48 of its rows are valid starts
                        nv = _NSTART - b * _M
                        nc.sync.dma_start(
                            o_d[b * _M : b * _M + nv, :],
                            ot[:nv, ns - 1, :],
                        )
                        ns -= 1
                    if ns > 0:
                        osrc = ot[:, :ns, :]
                        od1 = o_d[bb0 * _M : (bb0 + 1) * _M, :]
                        odst = dataclasses.replace(
                            od1,
                            ap=[list(od1.ap[0]), [_M * _W, ns], [1, _W]],
                        )
                        nc.sync.dma_start(odst, osrc)
    nc.compile()
    return nc


def _get_program():
    if not _NC_CACHE:
        _NC_CACHE.append(_build_program())
    return _NC_CACHE[0]


def kernel(images: np.ndarray) -> np.ndarray:
    import ml_dtypes
    from concourse.bass_utils import run_bass_kernel_spmd

    f8 = ml_dtypes.float8_e4m3
    images = np.asarray(images, dtype=np.float32)
    padded = np.pad(
        images, ((0, 0), (0, 0), (_RADIUS, _RADIUS), (_RADIUS, _RADIUS)),
        mode="reflect",
    )
    shards = padded.reshape(_NCORES, _PLANES * _HP, _WP)
    xs = np.zeros((_NCORES, _XROWS, _WP), dtype=f8)
    xs[:, : _PLANES * _HP, :] = _quantize_diffuse(shards, f8)
    w = _banded_weights()
    nc = _get_program()
    in_maps = [{"x": xs[c], "w": w} for c in range(_NCORES)]
    res = run_bass_kernel_spmd(nc, in_maps, list(range(_NCORES)))
    out = np.stack(
        [np.asarray(res.results[c]["o"]) for c in range(_NCORES)], axis=0
    )
    out = out[:, : _PLANES * _HP, :].astype(np.float32)
    out = out.reshape(_NCORES, _PLANES, _HP, _W)[:, :, : _H, :]
    return np.ascontiguousarray(
        out.reshape(_B, _C, _H, _W).astype(np.float32)
    )
